# revision 1
# baseline (speedup 1.0000x reference)
"""GAT (8-layer, 8-head) Trainium2 Bass kernel v2, 8-core SPMD.

Strategy (degree-partitioned edge layout; gather + reduce, no scatter matmuls):
- Host: add self-loops; sort nodes by in-degree (desc); permuted node blocks
  of 128 are degree-homogeneous. Deal blocks round-robin: block r ->
  core r%8, window r//8; table id of new-node q is
  (r%8)*2560 + (r//8)*128 + q%128.  Per (core, window): partition p =
  dst-local index, tile t = per-dst edge rank.  T[wl] = max degree over the
  8 blocks of window wl (static, shared by all cores).
- Device, per layer: fused [h|s] = x @ [W | W@A2] (bf16 PE), node-major
  256B table rows [h bf16 0:64 | s_src f32@32:40 | s_dst f32@40:48],
  AllGather -> TAB.  Per window: dma_gather rows by src (slot partition =
  dst-local), e = lrelu(s_src + s_dst[p]) where s_dst is a per-partition
  free-dim broadcast of the local table rows, ex = exp(e) written bf16 into
  gathered cols 64:72, R = [ex*h | ex] built in place, one tensor_reduce
  over the tile axis accumulates out[p] and z[p].  No dst gather, no
  one-hot, no PE scatter.
- Padding slots point at a sentinel row with s_src = -1e30 (=> ex = 0).
"""

import numpy as np
import ml_dtypes

# Persistent XLA compilation cache: run_bass_kernel_spmd re-traces a fresh
# closure per call, so without this every invocation recompiles the same HLO.
try:
    import jax
    jax.config.update("jax_compilation_cache_dir", "/tmp/jax_comp_cache")
    jax.config.update("jax_persistent_cache_min_entry_size_bytes", 0)
    jax.config.update("jax_persistent_cache_min_compile_time_secs", 0.0)
except Exception:
    pass
try:
    # drop the bass_exec ordering-effect: skips per-call effects-token sync
    jax.config.update("bass_fast_dispatch", True)
except Exception:
    pass

N_NODES = 20000
N_EDGES = 640000
L, H, C = 8, 8, 8
D = H * C  # 64
NEG_SLOPE = 0.2

NCORES = 8
WIN = 128
WPC = 20                  # windows per core
NSH = WIN * WPC           # 2560 nodes per shard
NPAD = NCORES * NSH       # 20480
SENT = NPAD               # sentinel table row
TROWS = NPAD + 1

_cache = {}
SIM1 = False              # single-core variant for TimelineSim (fake allgather)
GMAX = 1024               # indices per dma_gather call
GATH0 = False             # ablation: memset instead of gathers (timing probe)
COLL0 = False             # ablation: local DMA instead of AllGather (timing probe)


# ----------------------------------------------------------------------------
# Host preprocessing
# ----------------------------------------------------------------------------
def _prep_edges(edge_index):
    src = np.asarray(edge_index[0], dtype=np.int64).astype(np.int32)
    dst = np.asarray(edge_index[1], dtype=np.int64).astype(np.int32)
    loops = np.arange(N_NODES, dtype=np.int32)
    src = np.concatenate([src, loops])
    dst = np.concatenate([dst, loops])

    deg = np.bincount(dst, minlength=N_NODES)          # incl. self loop
    order = np.argsort(-deg, kind="stable")            # orig ids, deg desc
    newid = np.empty(N_NODES, np.int32)
    newid[order] = np.arange(N_NODES, dtype=np.int32)

    q_all = np.arange(NPAD, dtype=np.int32)
    r_all = q_all // 128
    tab_of_q = (r_all % NCORES) * NSH + (r_all // NCORES) * 128 + (q_all % 128)

    degq = deg[order]                                   # deg sorted desc
    T = tuple(int(degq[w * 1024]) for w in range(WPC))
    assert all(t > 0 for t in T)
    # group adjacent windows, padding each group's windows to the group max;
    # a group's windows share one set of DVE/ACT ops in the program
    groups = []                                          # (w0, wcount, Tg)
    w0 = 0
    while w0 < WPC:
        Tg = T[w0]
        w1 = w0 + 1
        while w1 < WPC and Tg * (w1 + 1 - w0) - sum(T[w0:w1 + 1]) <= 8:
            w1 += 1
        groups.append((w0, w1 - w0, Tg))
        w0 = w1
    groups = tuple(groups)
    Tpad = []
    for (_, wc, Tg) in groups:
        Tpad += [Tg] * wc
    off = np.zeros(WPC, np.int64)
    off[1:] = np.cumsum(Tpad[:-1])
    nslot = int(sum(Tpad)) * 128

    qd = newid[dst]
    ts = tab_of_q[newid[src]].astype(np.int16)

    o2 = np.argsort(qd, kind="stable")
    qds = qd[o2]
    newgrp = np.empty(len(qds), bool)
    newgrp[0] = True
    newgrp[1:] = qds[1:] != qds[:-1]
    gidx = np.cumsum(newgrp) - 1
    gstart = np.flatnonzero(newgrp)
    t_rank = np.arange(len(qds), dtype=np.int64) - gstart[gidx]

    core_s = (qds // 128) % NCORES
    wl_s = qds // 1024
    p_s = qds % 128
    pos = (off[wl_s] + t_rank) * 128 + p_s

    srcslot = np.full((NCORES, nslot), SENT, dtype=np.int16)
    srcslot[core_s, pos] = ts[o2]
    # wrap16: flat k -> [k % 16, k // 16]
    srci = np.stack([srcslot[c].reshape(-1, 16).T.copy()
                     for c in range(NCORES)])            # [8, 16, nslot/16]

    return groups, srci, order


# ----------------------------------------------------------------------------
# Bass program
# ----------------------------------------------------------------------------
def _build(groups):
    import concourse.bass as bass
    import concourse.tile as tile
    import concourse.mybir as mybir
    from concourse import bacc
    from contextlib import ExitStack

    f32 = mybir.dt.float32
    bf16 = mybir.dt.bfloat16
    i16 = mybir.dt.int16
    Alu = mybir.AluOpType
    Act = mybir.ActivationFunctionType
    Ax = mybir.AxisListType

    Tpad = []
    for (_, wc, Tg) in groups:
        Tpad += [Tg] * wc
    GTmax = max(wc * Tg for (_, wc, Tg) in groups)   # tiles in largest group
    off = [0]
    for t in Tpad[:-1]:
        off.append(off[-1] + t)
    nslot = sum(Tpad) * 128
    n16 = nslot // 16

    nc = bacc.Bacc("TRN2", target_bir_lowering=False, debug=False,
                   num_devices=1 if SIM1 else NCORES)

    # single packed input: [xsh bf16 | srci i16 | wts bf16 | bias f32] bytes
    u8 = mybir.dt.uint8
    O_X = 0
    O_SRCI = O_X + NSH * D * 2
    O_WTS = O_SRCI + 16 * n16 * 2
    O_BIAS = O_WTS + 64 * L * 80 * 2
    NB = O_BIAS + L * 64 * 4
    t_pack = nc.dram_tensor("pack", [NB], u8, kind="ExternalInput")
    t_out = nc.dram_tensor("out", [NSH, D], bf16, kind="ExternalOutput")

    with tile.TileContext(nc) as tc, ExitStack() as ctx:
        cpool = ctx.enter_context(tc.tile_pool(name="const", bufs=1))
        wpool = ctx.enter_context(tc.tile_pool(name="work", bufs=2))
        gpool = ctx.enter_context(tc.tile_pool(name="gath", bufs=2))
        epool = ctx.enter_context(tc.tile_pool(name="edge", bufs=2))
        dram = ctx.enter_context(tc.tile_pool(name="dram", bufs=1, space="DRAM"))
        psT = ctx.enter_context(tc.tile_pool(name="psT", bufs=2, space="PSUM"))
        psA = ctx.enter_context(tc.tile_pool(name="psA", bufs=2, space="PSUM"))

        # persistent SBUF
        sb_xb = cpool.tile([128, WPC, D], bf16)
        sb_x = cpool.tile([128, WPC, D], f32)
        sb_srci = cpool.tile([128, n16], i16)
        sb_wts = cpool.tile([64, L, 80], bf16)
        sb_bias = cpool.tile([1, L * 64], f32)
        sb_brep = cpool.tile([128, L * 64], f32)
        sb_ident = cpool.tile([128, 128], f32)

        ap_x = t_pack.ap()[O_X:O_SRCI].bitcast(bf16)
        ap_srci = t_pack.ap()[O_SRCI:O_WTS].bitcast(i16)
        ap_wts = t_pack.ap()[O_WTS:O_BIAS].bitcast(bf16)
        ap_bias = t_pack.ap()[O_BIAS:NB].bitcast(f32)
        nc.sync.dma_start(sb_xb[:],
                          ap_x.rearrange("(t p c) -> p t c", p=128, c=D))
        nc.vector.tensor_copy(sb_x[:], sb_xb[:])
        for k in range(8):
            nc.sync.dma_start(sb_srci[16 * k:16 * (k + 1), :],
                              ap_srci.rearrange("(p c) -> p c", p=16))
        nc.sync.dma_start(sb_wts[:],
                          ap_wts.rearrange("(p l c) -> p l c", p=64, c=80))
        nc.sync.dma_start(sb_bias[:], ap_bias.rearrange("(p c) -> p c", p=1))
        nc.gpsimd.partition_broadcast(sb_brep[:], sb_bias[:])
        # identity = affine_select(p - j == 0 ? 1 : 0)
        nc.vector.memset(sb_ident[:], 1.0)
        nc.gpsimd.affine_select(sb_ident[:], sb_ident[:], pattern=[[-1, 128]],
                                compare_op=Alu.is_equal, fill=0.0,
                                base=0, channel_multiplier=1)

        TAB = dram.tile([TROWS, 128], bf16)
        STAGE = dram.tile([NSH, 128], bf16)

        # STAGE bf16 cols 96:128 never produced; zero once (keeps sim finite)
        zjunk = cpool.tile([128, WPC, 32], bf16)
        nc.vector.memset(zjunk[:], 0.0)
        nc.sync.dma_start(
            STAGE[:, 96:128].rearrange("(t p) c -> p t c", p=128), zjunk[:])

        # sentinel row: h=0, s_src=-1e30 => ex = 0 for padding slots
        sent = cpool.tile([1, 128], bf16)
        nc.vector.memset(sent[:], 0.0)
        nc.vector.memset(sent[:].bitcast(f32)[:, 32:40], -1e30)
        nc.sync.dma_start(TAB[SENT:SENT + 1, :], sent[:])

        reg_cache = {}

        def nreg(n):
            if n not in reg_cache:
                reg_cache[n] = nc.gpsimd.to_reg(n)
            return reg_cache[n]

        for l in range(L):
            # ---------------- phase A: projection + table ------------------
            xT = wpool.tile([64, NSH], bf16, tag="xT")
            for t0 in range(0, WPC, 4):
                pt = psT.tile([64, 4, 128], f32, tag="psTa")
                for t in range(t0, t0 + 4):
                    nc.tensor.transpose(pt[:, t - t0, :], sb_x[:, t, :],
                                        sb_ident[:])
                nc.scalar.copy(
                    xT[:, t0 * 128:(t0 + 4) * 128].rearrange(
                        "p (t c) -> p t c", t=4), pt[:])

            hsT = wpool.tile([80, NSH], f32, tag="hsT")
            for k0 in range(0, NSH, 512):
                k1 = min(k0 + 512, NSH)
                ph = psA.tile([80, k1 - k0], f32, tag="psA")
                nc.tensor.matmul(ph[:], lhsT=sb_wts[:, l, :],
                                 rhs=xT[:, k0:k1], start=True, stop=True)
                nc.scalar.copy(hsT[:, k0:k1], ph[:])

            tabsb = wpool.tile([128, WPC, 128], bf16, tag="tabsb")
            for t0 in range(0, WPC, 4):
                pt = psT.tile([128, 4, 80], f32, tag="psTb")
                for t in range(t0, t0 + 4):
                    nc.tensor.transpose(pt[:, t - t0, :],
                                        hsT[:, t * 128:(t + 1) * 128],
                                        sb_ident[:80, :80])
                nc.scalar.copy(tabsb[:, t0:t0 + 4, 0:64], pt[:, :, 0:64])
                nc.vector.tensor_copy(
                    tabsb[:, t0:t0 + 4, :].bitcast(f32)[:, :, 32:48],
                    pt[:, :, 64:80])

            nc.sync.dma_start(
                STAGE[:, 0:96].rearrange("(t p) c -> p t c", p=128),
                tabsb[:, :, 0:96])
            if SIM1 or COLL0:
                for c in range(NCORES):
                    nc.sync.dma_start(TAB[c * NSH:(c + 1) * NSH, :], STAGE[:])
            else:
                nc.gpsimd.collective_compute(
                    "AllGather", Alu.bypass,
                    replica_groups=[list(range(NCORES))],
                    ins=[STAGE[:].opt()],
                    outs=[TAB[0:NPAD, :].opt()],
                )

            # ---------------- phase B: edges, per window group -------------
            layerbuf = wpool.tile([128, WPC, 72], f32, tag="layerbuf")
            for (w0, wc, Tg) in groups:
                tiles = wc * Tg
                nW = 128 * tiles
                i0 = 128 * off[w0]
                vs = gpool.tile([128, GTmax, 128], bf16, tag="vs")
                if GATH0:
                    nc.vector.memset(vs[:, 0:tiles, :], 0.01)
                else:
                    for j0 in range(0, nW, GMAX):
                        j1 = min(j0 + GMAX, nW)
                        n = j1 - j0
                        nc.gpsimd.dma_gather(
                            out_ap=vs[:, j0 // 128:j1 // 128, :], in_ap=TAB[:],
                            idxs_ap=sb_srci[:, (i0 + j0) // 16:(i0 + j1) // 16],
                            num_idxs=n, num_idxs_reg=nreg(n), elem_size=128)

                e = epool.tile([128, GTmax, 8], f32, tag="e")
                nc.vector.tensor_tensor(
                    e[:, :tiles].rearrange("p (w t) c -> p w t c", w=wc),
                    vs[:, :tiles, :].bitcast(f32)[:, :, 32:40]
                        .rearrange("p (w t) c -> p w t c", w=wc),
                    tabsb[:, w0:w0 + wc, :].bitcast(f32)[:, :, 40:48]
                        .unsqueeze(2).broadcast_to([128, wc, Tg, 8]),
                    Alu.add)
                nc.vector.scalar_tensor_tensor(
                    e[:, :tiles], e[:, :tiles], NEG_SLOPE, e[:, :tiles],
                    op0=Alu.mult, op1=Alu.max)
                # ex -> bf16, written into gathered cols 64:72
                nc.scalar.activation(vs[:, :tiles, 64:72], e[:, :tiles],
                                     Act.Exp)
                # R = [h*ex | ex] in place
                nc.vector.tensor_tensor(
                    vs[:, :tiles, 0:64].rearrange("p t (h c) -> p t h c", h=8),
                    vs[:, :tiles, 0:64].rearrange("p t (h c) -> p t h c", h=8),
                    vs[:, :tiles, 64:72].unsqueeze(3).broadcast_to(
                        [128, tiles, 8, 8]),
                    Alu.mult)
                # out[p, w, 0:72] = sum over t within each window
                nc.vector.tensor_reduce(
                    layerbuf[:, w0:w0 + wc, :],
                    vs[:, :tiles, 0:72].rearrange("p (w t) c -> p w c t", w=wc),
                    axis=Ax.X, op=Alu.add)

            # ---------------- finals: x = out/(z+eps) + b ------------------
            zi = epool.tile([128, WPC, 8], f32, tag="zi")
            nc.vector.tensor_scalar_add(zi[:], layerbuf[:, :, 64:72], 1e-16)
            nc.vector.reciprocal(zi[:], zi[:])
            nc.vector.tensor_tensor(
                sb_x[:].rearrange("p w (h c) -> p w h c", h=8),
                layerbuf[:, :, 0:64].rearrange("p w (h c) -> p w h c", h=8),
                zi[:].unsqueeze(3).broadcast_to([128, WPC, 8, 8]),
                Alu.mult)
            nc.vector.tensor_tensor(
                sb_x[:], sb_x[:],
                sb_brep[:, l * 64:(l + 1) * 64].unsqueeze(1)
                    .broadcast_to([128, WPC, 64]),
                Alu.add)

        ob = cpool.tile([128, WPC, D], bf16)
        nc.vector.tensor_copy(ob[:], sb_x[:])
        nc.sync.dma_start(t_out.ap().rearrange("(t p) c -> p t c", p=128),
                          ob[:])

    nc.finalize()
    return nc


def _get_program(groups):
    key = (groups, SIM1, GMAX, GATH0, COLL0)
    if key not in _cache:
        nc = _build(groups)
        # the jaxpr lowering re-serializes the module on every call; the
        # program is immutable after finalize(), so serialize once
        try:
            blob = nc.to_json_bytes()
            nc.to_json_bytes = lambda _b=blob: _b
        except Exception:
            pass
        _cache[key] = nc
    return _cache[key]


# ----------------------------------------------------------------------------
# Entry point
# ----------------------------------------------------------------------------
def make_program_and_inputs(x, edge_index, Ws, att_src, att_dst, biases):
    x = np.asarray(x, dtype=np.float32)
    Ws = np.asarray(Ws, dtype=np.float32)
    att_src = np.asarray(att_src, dtype=np.float32)
    att_dst = np.asarray(att_dst, dtype=np.float32)
    biases = np.asarray(biases, dtype=np.float32)

    groups, srci, order = _prep_edges(edge_index)
    nc = _get_program(groups)

    # per-core x shards in (window, pos) order
    m = np.arange(NSH)
    xsh = []
    for c in range(NCORES):
        q = ((m // 128) * NCORES + c) * 128 + (m % 128)
        xc = np.zeros((NSH, D), np.float32)
        real = q < N_NODES
        xc[real] = x[order[q[real]]]
        xsh.append(xc.astype(ml_dtypes.bfloat16))

    a2 = np.zeros((64, L, 16), np.float32)
    for l in range(L):
        for h in range(H):
            a2[h * C:(h + 1) * C, l, h] = att_src[l, h]
            a2[h * C:(h + 1) * C, l, 8 + h] = att_dst[l, h]
    wts = np.zeros((64, L, 80), np.float32)
    for l in range(L):
        wts[:, l, 0:64] = Ws[l]
        wts[:, l, 64:80] = Ws[l] @ a2[:, l, :]
    wts = wts.astype(ml_dtypes.bfloat16)
    bias = biases.reshape(1, L * 64).copy()

    in_maps = []
    for c in range(NCORES):
        blob = b"".join([xsh[c].tobytes(),
                         np.ascontiguousarray(srci[c]).tobytes(),
                         wts.tobytes(), bias.tobytes()])
        in_maps.append(dict(pack=np.frombuffer(blob, np.uint8).copy()))

    # output reassembly indices: out_full[order[q]] = res[core(q)][m(q)]
    q = np.arange(N_NODES)
    core_q = (q // 128) % NCORES
    m_q = (q // 1024) * 128 + (q % 128)
    return nc, in_maps, (order, core_q, m_q)


def assemble_output(res, meta):
    order, core_q, m_q = meta
    shards = [np.asarray(res.results[c]["out"]).astype(np.float32)
              for c in range(NCORES)]
    allout = np.stack(shards)                          # [8, 2560, 64]
    out = np.empty((N_NODES, D), np.float32)
    out[order] = allout[core_q, m_q]
    return out


def kernel(x, edge_index, Ws, att_src, att_dst, biases):
    from concourse.bass_utils import run_bass_kernel_spmd

    nc, in_maps, meta = make_program_and_inputs(
        x, edge_index, Ws, att_src, att_dst, biases)
    res = run_bass_kernel_spmd(nc, in_maps, core_ids=list(range(NCORES)))
    return assemble_output(res, meta)



# revision 9
# speedup vs baseline: 37.0643x; 37.0643x over previous
"""GAT (8-layer, 8-head) Trainium2 Bass kernel v3, 8-core SPMD.

Strategy (degree-partitioned edge layout; gather + reduce, no scatter matmuls):
- Host: add self-loops; sort nodes by in-degree (desc); blocks of 128 sorted
  nodes are degree-homogeneous.  Snake-deal blocks: block r -> window r//8,
  core (r%8 on even windows, 7-r%8 on odd) so per-core real edge counts
  balance.  Per (core, window): partition p = dst-local index, tile t =
  per-dst edge rank.  T[w] = max degree in window w (static, shared).
- Device, per layer: per-window pipeline: dma_gather 256B table rows by src
  (slot partition = dst-local), e = lrelu(s_src + s_dst[p]), ex = exp(e)
  bf16 into gathered cols 64:72, R = [ex*h | ex] in place, tensor_reduce
  over tiles -> out/z, finals -> x, then immediately project the window with
  the NEXT layer's fused [W | W@A2] and stage its table rows.  The table
  AllGather is split in two (windows [0,SPLIT) early, rest at layer end) and
  double-buffered (TAB0/TAB1) so it overlaps the tail windows' gathers.
- Padding slots point at a sentinel row with s_src = -1e30 (=> ex = 0).
  Trailing padding of each window's final gather chunk is trimmed per-core:
  those idxs are -1 (ucode skips them; per-core valid counts ride in the
  input pack and feed num_idxs_reg via value_load), and the possibly-skipped
  tail tiles are pre-memset to the sentinel pattern.
"""

import numpy as np
import ml_dtypes

# Persistent XLA compilation cache: run_bass_kernel_spmd re-traces a fresh
# closure per call, so without this every invocation recompiles the same HLO.
try:
    import jax
    jax.config.update("jax_compilation_cache_dir", "/tmp/jax_comp_cache")
    jax.config.update("jax_persistent_cache_min_entry_size_bytes", 0)
    jax.config.update("jax_persistent_cache_min_compile_time_secs", 0.0)
except Exception:
    pass
try:
    jax.config.update("bass_fast_dispatch", True)
except Exception:
    pass

N_NODES = 20000
N_EDGES = 640000
L, H, C = 8, 8, 8
D = H * C  # 64
NEG_SLOPE = 0.2

NCORES = 8
WIN = 128
WPC = 20                  # windows per core
NSH = WIN * WPC           # 2560 nodes per shard
NPAD = NCORES * NSH       # 20480
SENT = NPAD               # sentinel table row
TROWS = NPAD + 1

_cache = {}
SIM1 = False              # single-core variant (fake allgather)
GMAX = 1024               # idxs per dma_gather call (HW max; larger wedges SWDGE)
TRIM = True               # per-core trailing-padding trim via -1 idxs
REGPROBE = False          # use count regs even with TRIM off (isolation probe)
AGSPLIT = True            # split table AllGather (early half mid-layer)
COLL0 = False             # ablation: local DMA instead of AllGather


def _chunks(nW):
    """Chunk a window's nW slots: remainder first so the FINAL chunk is a
    full GMAX (maximizes the per-core trailing trim headroom)."""
    rem = nW % GMAX
    out = []
    if rem:
        out.append((0, rem))
    for a in range(rem, nW, GMAX):
        out.append((a, a + GMAX))
    return out


def _split_of(groups):
    bounds = [w0 + wc for (w0, wc, _) in groups]
    return min(bounds, key=lambda b: abs(b - (WPC - 4)))


# ----------------------------------------------------------------------------
# Host preprocessing
# ----------------------------------------------------------------------------
def _prep_edges(edge_index):
    src = np.asarray(edge_index[0], dtype=np.int64).astype(np.int32)
    dst = np.asarray(edge_index[1], dtype=np.int64).astype(np.int32)
    loops = np.arange(N_NODES, dtype=np.int32)
    src = np.concatenate([src, loops])
    dst = np.concatenate([dst, loops])

    deg = np.bincount(dst, minlength=N_NODES)          # incl. self loop
    order = np.argsort(-deg, kind="stable")            # orig ids, deg desc
    newid = np.empty(N_NODES, np.int32)
    newid[order] = np.arange(N_NODES, dtype=np.int32)

    degq = deg[order]                                   # deg sorted desc
    T = tuple(int(degq[w * 1024]) for w in range(WPC))
    assert all(t > 0 for t in T)
    # groups of adjacent equal-T windows (zero tile padding) sharing DVE ops
    groups = []                                          # (w0, wcount, Tg)
    w0 = 0
    while w0 < WPC:
        Tg = T[w0]
        w1 = w0 + 1
        while w1 < WPC and T[w1] == Tg:
            w1 += 1
        groups.append((w0, w1 - w0, Tg))
        w0 = w1
    groups = tuple(groups)
    split = _split_of(groups)
    off = np.zeros(WPC, np.int64)
    off[1:] = np.cumsum(T[:-1])
    nslot = int(sum(T)) * 128

    # greedy (LPT-ish) block dealing: per window, heaviest block to the
    # currently lightest core; perm[w, j] = core holding block 8w+j
    degq_pad = np.concatenate(
        [degq, np.zeros(NPAD - N_NODES, degq.dtype)])
    bsum = degq_pad.reshape(NCORES * WPC, 128).sum(1)
    loads = np.zeros(NCORES, np.int64)
    perm = np.zeros((WPC, NCORES), np.int32)
    for w in range(WPC):
        bs = bsum[NCORES * w:NCORES * (w + 1)]
        for bi in np.argsort(-bs):
            c = int(np.argmin(loads))
            perm[w, bi] = c
            loads[c] += bs[bi]
    iperm = np.zeros((WPC, NCORES), np.int32)   # iperm[w, c] = block j
    for w in range(WPC):
        iperm[w, perm[w]] = np.arange(NCORES)

    # table ids: halves contiguous (AllGather splits at window `split`)
    q_all = np.arange(NPAD, dtype=np.int64)
    r_all = q_all // 128
    w_all = r_all // NCORES
    j_all = r_all % NCORES
    c_all = perm[w_all, j_all].astype(np.int64)
    p_all = q_all % 128
    B0 = NCORES * split * 128
    tab_of_q = np.where(
        w_all < split,
        c_all * (split * 128) + w_all * 128 + p_all,
        B0 + c_all * ((WPC - split) * 128) + (w_all - split) * 128 + p_all,
    ).astype(np.int32)

    qd = newid[dst]
    ts = tab_of_q[newid[src]].astype(np.int16)

    o2 = np.argsort(qd, kind="stable")
    qds = qd[o2]
    newgrp = np.empty(len(qds), bool)
    newgrp[0] = True
    newgrp[1:] = qds[1:] != qds[:-1]
    gidx = np.cumsum(newgrp) - 1
    gstart = np.flatnonzero(newgrp)
    t_rank = np.arange(len(qds), dtype=np.int64) - gstart[gidx]

    r_s = qds // 128
    wl_s = r_s // NCORES
    j_s = r_s % NCORES
    core_s = perm[wl_s, j_s]
    p_s = qds % 128
    pos = (off[wl_s] + t_rank) * 128 + p_s

    srcslot = np.full((NCORES, nslot), SENT, dtype=np.int16)
    srcslot[core_s, pos] = ts[o2]

    # per-core trailing-padding trim: within each window's FINAL gather
    # chunk, replace the trailing run of sentinels with -1 (ucode skips
    # trailing negatives); per-core valid counts ship in the pack.
    counts = np.zeros((NCORES, WPC), np.int32)
    gt = [0] * WPC
    for w in range(WPC):
        nW = T[w] * 128
        base = int(off[w]) * 128
        last_len = min(GMAX, nW)
        tmax = 0
        for c in range(NCORES):
            arr = srcslot[c, base:base + nW]
            nz = np.flatnonzero(arr != SENT)
            run = nW if len(nz) == 0 else nW - 1 - int(nz[-1])
            # keep >=128 valid idxs: an all-negative gather call is invalid
            trim = min(run, last_len - 128) if TRIM else 0
            if trim > 0:
                arr[nW - trim:nW] = -1
            counts[c, w] = last_len - trim
            tmax = max(tmax, trim)
        gt[w] = (tmax + 127) // 128
    gt = tuple(gt)

    # wrap16: flat k -> [k % 16, k // 16]
    srci = np.stack([srcslot[c].reshape(-1, 16).T.copy()
                     for c in range(NCORES)])            # [8, 16, nslot/16]

    return groups, gt, srci, counts, order, perm, iperm


# ----------------------------------------------------------------------------
# Bass program
# ----------------------------------------------------------------------------
def _build(groups, gt):
    import concourse.bass as bass
    import concourse.tile as tile
    import concourse.mybir as mybir
    from concourse import bacc
    from contextlib import ExitStack

    f32 = mybir.dt.float32
    bf16 = mybir.dt.bfloat16
    i16 = mybir.dt.int16
    i32 = mybir.dt.int32
    Alu = mybir.AluOpType
    Act = mybir.ActivationFunctionType
    Ax = mybir.AxisListType

    T = []
    for (_, wc, Tg) in groups:
        T += [Tg] * wc
    GTmax = max(wc * Tg for (_, wc, Tg) in groups)   # tiles in largest group
    off = [0]
    for t in T[:-1]:
        off.append(off[-1] + t)
    nslot = sum(T) * 128
    n16 = nslot // 16
    split = _split_of(groups)
    B0 = NCORES * split * 128
    NS2 = WPC - split

    nc = bacc.Bacc("TRN2", target_bir_lowering=False, debug=False,
                   num_devices=1 if SIM1 else NCORES)

    # packed input: [xsh bf16 | srci i16 | wts bf16 | bias f32 | counts i32]
    u8 = mybir.dt.uint8
    O_X = 0
    O_SRCI = O_X + NSH * D * 2
    O_WTS = O_SRCI + 16 * n16 * 2
    O_BIAS = O_WTS + 64 * L * 80 * 2
    O_CNT = O_BIAS + L * 64 * 4
    NB = O_CNT + WPC * 4
    t_pack = nc.dram_tensor("pack", [NB], u8, kind="ExternalInput")
    t_out = nc.dram_tensor("out", [NSH, D], bf16, kind="ExternalOutput")

    with tile.TileContext(nc) as tc, ExitStack() as ctx:
        cpool = ctx.enter_context(tc.tile_pool(name="const", bufs=1))
        wpool = ctx.enter_context(tc.tile_pool(name="work", bufs=2))
        gpool = ctx.enter_context(tc.tile_pool(name="gath", bufs=2))
        epool = ctx.enter_context(tc.tile_pool(name="edge", bufs=2))
        dram = ctx.enter_context(tc.tile_pool(name="dram", bufs=1, space="DRAM"))
        psT = ctx.enter_context(tc.tile_pool(name="psT", bufs=2, space="PSUM"))
        psA = ctx.enter_context(tc.tile_pool(name="psA", bufs=2, space="PSUM"))

        # persistent SBUF
        sb_xb = cpool.tile([128, WPC, D], bf16)
        sb_x = cpool.tile([128, WPC, D], f32)
        sb_srci = cpool.tile([128, n16], i16)
        sb_wts = cpool.tile([64, L, 80], bf16)
        sb_bias = cpool.tile([1, L * 64], f32)
        sb_brep = cpool.tile([128, L * 64], f32)
        sb_ident = cpool.tile([128, 128], f32)
        sb_cnt = cpool.tile([1, WPC], i32)
        tabsb = cpool.tile([128, WPC, 128], bf16)

        ap_x = t_pack.ap()[O_X:O_SRCI].bitcast(bf16)
        ap_srci = t_pack.ap()[O_SRCI:O_WTS].bitcast(i16)
        ap_wts = t_pack.ap()[O_WTS:O_BIAS].bitcast(bf16)
        ap_bias = t_pack.ap()[O_BIAS:O_CNT].bitcast(f32)
        ap_cnt = t_pack.ap()[O_CNT:NB].bitcast(i32)
        nc.sync.dma_start(sb_xb[:],
                          ap_x.rearrange("(t p c) -> p t c", p=128, c=D))
        nc.vector.tensor_copy(sb_x[:], sb_xb[:])
        for k in range(8):
            nc.sync.dma_start(sb_srci[16 * k:16 * (k + 1), :],
                              ap_srci.rearrange("(p c) -> p c", p=16))
        nc.sync.dma_start(sb_wts[:],
                          ap_wts.rearrange("(p l c) -> p l c", p=64, c=80))
        nc.sync.dma_start(sb_bias[:], ap_bias.rearrange("(p c) -> p c", p=1))
        nc.sync.dma_start(sb_cnt[:], ap_cnt.rearrange("(p c) -> p c", p=1))
        nc.gpsimd.partition_broadcast(sb_brep[:], sb_bias[:])
        # identity = affine_select(p - j == 0 ? 1 : 0)
        nc.vector.memset(sb_ident[:], 1.0)
        nc.gpsimd.affine_select(sb_ident[:], sb_ident[:], pattern=[[-1, 128]],
                                compare_op=Alu.is_equal, fill=0.0,
                                base=0, channel_multiplier=1)

        # per-window valid-count registers for the final gather chunks
        cnt_val = [None] * WPC
        if TRIM or REGPROBE:
            for w in range(WPC):
                r = nc.gpsimd.alloc_register(f"cntreg{w}")
                nc.gpsimd.reg_load(r, sb_cnt[0:1, w:w + 1])
                cnt_val[w] = r

        TAB = [dram.tile([TROWS, 128], bf16, name=f"TAB{i}") for i in (0, 1)]
        STA = dram.tile([split * 128, 128], bf16)
        STB = dram.tile([NS2 * 128, 128], bf16)

        # cols 96:128 of table rows are never produced; zero once
        zjunk = cpool.tile([128, WPC, 32], bf16)
        nc.vector.memset(zjunk[:], 0.0)
        nc.sync.dma_start(
            STA[:, 96:128].rearrange("(t p) c -> p t c", p=128),
            zjunk[:, 0:split, :])
        nc.sync.dma_start(
            STB[:, 96:128].rearrange("(t p) c -> p t c", p=128),
            zjunk[:, 0:NS2, :])

        # sentinel row: h=0, s_src=-1e30 => ex = 0 for padding slots
        sent = cpool.tile([1, 128], bf16)
        nc.vector.memset(sent[:], 0.0)
        nc.vector.memset(sent[:].bitcast(f32)[:, 32:40], -1e30)
        nc.sync.dma_start(TAB[0][SENT:SENT + 1, :], sent[:])
        nc.sync.dma_start(TAB[1][SENT:SENT + 1, :], sent[:])

        reg_cache = {}

        def nreg(n):
            if n not in reg_cache:
                reg_cache[n] = nc.gpsimd.to_reg(n)
            return reg_cache[n]

        def project_window(w, wl):
            """tabsb[:, w] <- table rows for layer `wl` from sb_x[:, w]."""
            ptx = psT.tile([64, 128], f32, tag="ptx")
            nc.tensor.transpose(ptx[:], sb_x[:, w, :], sb_ident[:])
            xTw = wpool.tile([64, 128], bf16, tag="xTw")
            nc.scalar.copy(xTw[:], ptx[:])
            ph = psA.tile([80, 128], f32, tag="ph")
            nc.tensor.matmul(ph[:], lhsT=sb_wts[:, wl, :], rhs=xTw[:],
                             start=True, stop=True)
            hsw = wpool.tile([80, 128], f32, tag="hsw")
            nc.scalar.copy(hsw[:], ph[:])
            ptb = psT.tile([128, 80], f32, tag="ptb")
            nc.tensor.transpose(ptb[:], hsw[:], sb_ident[:80, :80])
            nc.scalar.copy(tabsb[:, w, 0:64], ptb[:, 0:64])
            nc.vector.tensor_copy(
                tabsb[:, w:w + 1, :].bitcast(f32)[:, :, 32:48],
                ptb[:, 64:80].unsqueeze(1))

        def stage_window(w):
            if w < split:
                dst = STA[w * 128:(w + 1) * 128, 0:96]
            else:
                ws = w - split
                dst = STB[ws * 128:(ws + 1) * 128, 0:96]
            nc.sync.dma_start(dst.rearrange("(t p) c -> p t c", p=128),
                              tabsb[:, w:w + 1, 0:96])

        def allgather(src, dst_lo, dst_hi, tabw):
            if SIM1 or COLL0:
                sz = (dst_hi - dst_lo) // NCORES
                for c in range(NCORES):
                    nc.sync.dma_start(
                        tabw[dst_lo + c * sz:dst_lo + (c + 1) * sz, :], src[:])
            else:
                nc.gpsimd.collective_compute(
                    "AllGather", Alu.bypass,
                    replica_groups=[list(range(NCORES))],
                    ins=[src[:].opt()],
                    outs=[tabw[dst_lo:dst_hi, :].opt()],
                )

        # ---------------- prologue: table for layer 0 ----------------------
        for w in range(WPC):
            project_window(w, 0)
            stage_window(w)
        allgather(STA, 0, B0, TAB[0])
        allgather(STB, B0, NPAD, TAB[0])

        # ---------------- layers -------------------------------------------
        for l in range(L):
            TABr = TAB[l % 2]
            TABw = TAB[(l + 1) % 2]
            layerbuf = wpool.tile([128, WPC, 72], f32, tag="layerbuf")
            for (w0, wc, Tg) in groups:
                tiles = wc * Tg
                vs = gpool.tile([128, GTmax, 128], bf16, tag="vs")
                # sentinel-pattern guard over possibly-skipped tail tiles
                for w in range(w0, w0 + wc):
                    if gt[w]:
                        a = (w - w0 + 1) * Tg - gt[w]
                        b = (w - w0 + 1) * Tg
                        nc.vector.memset(vs[:, a:b, :], 0.0)
                        nc.vector.memset(
                            vs[:, a:b, :].bitcast(f32)[:, :, 32:40], -1e30)
                # per-window gather calls (trailing -1s only in final chunk)
                for w in range(w0, w0 + wc):
                    nW = Tg * 128
                    i0 = 128 * off[w]
                    tb = (w - w0) * Tg
                    for (j0, j1) in _chunks(nW):
                        n = j1 - j0
                        reg = cnt_val[w] if ((TRIM or REGPROBE) and j1 == nW) else nreg(n)
                        nc.gpsimd.dma_gather(
                            out_ap=vs[:, tb + j0 // 128:tb + j1 // 128, :],
                            in_ap=TABr[:],
                            idxs_ap=sb_srci[:, (i0 + j0) // 16:(i0 + j1) // 16],
                            num_idxs=n, num_idxs_reg=reg, elem_size=128)

                e = epool.tile([128, GTmax, 8], f32, tag="e")
                nc.vector.tensor_tensor(
                    e[:, :tiles].rearrange("p (w t) c -> p w t c", w=wc),
                    vs[:, :tiles, :].bitcast(f32)[:, :, 32:40]
                        .rearrange("p (w t) c -> p w t c", w=wc),
                    tabsb[:, w0:w0 + wc, :].bitcast(f32)[:, :, 40:48]
                        .unsqueeze(2).broadcast_to([128, wc, Tg, 8]),
                    Alu.add)
                nc.vector.scalar_tensor_tensor(
                    e[:, :tiles], e[:, :tiles], NEG_SLOPE, e[:, :tiles],
                    op0=Alu.mult, op1=Alu.max)
                nc.scalar.activation(vs[:, :tiles, 64:72], e[:, :tiles],
                                     Act.Exp)
                nc.vector.tensor_tensor(
                    vs[:, :tiles, 0:64].rearrange("p t (h c) -> p t h c", h=8),
                    vs[:, :tiles, 0:64].rearrange("p t (h c) -> p t h c", h=8),
                    vs[:, :tiles, 64:72].unsqueeze(3).broadcast_to(
                        [128, tiles, 8, 8]),
                    Alu.mult)
                nc.vector.tensor_reduce(
                    layerbuf[:, w0:w0 + wc, :],
                    vs[:, :tiles, 0:72].rearrange("p (w t) c -> p w c t", w=wc),
                    axis=Ax.X, op=Alu.add)

                # finals for this group: x = out/(z+eps) + b
                zi = epool.tile([128, wc, 8], f32, tag=f"zi{wc}")
                nc.vector.tensor_scalar_add(
                    zi[:], layerbuf[:, w0:w0 + wc, 64:72], 1e-16)
                nc.vector.reciprocal(zi[:], zi[:])
                nc.vector.tensor_tensor(
                    sb_x[:, w0:w0 + wc].rearrange("p w (h c) -> p w h c", h=8),
                    layerbuf[:, w0:w0 + wc, 0:64]
                        .rearrange("p w (h c) -> p w h c", h=8),
                    zi[:].unsqueeze(3).broadcast_to([128, wc, 8, 8]),
                    Alu.mult)
                nc.vector.tensor_tensor(
                    sb_x[:, w0:w0 + wc], sb_x[:, w0:w0 + wc],
                    sb_brep[:, l * 64:(l + 1) * 64].unsqueeze(1)
                        .broadcast_to([128, wc, 64]),
                    Alu.add)

                if l < L - 1:
                    for w in range(w0, w0 + wc):
                        project_window(w, l + 1)
                        stage_window(w)
                    if w0 + wc == split:
                        allgather(STA, 0, B0, TABw)
            if l < L - 1:
                allgather(STB, B0, NPAD, TABw)

        ob = cpool.tile([128, WPC, D], bf16)
        nc.vector.tensor_copy(ob[:], sb_x[:])
        nc.sync.dma_start(t_out.ap().rearrange("(t p) c -> p t c", p=128),
                          ob[:])

    nc.finalize()
    return nc


def _get_program(groups, gt):
    key = (groups, gt, SIM1, GMAX, COLL0, TRIM, REGPROBE)
    if key not in _cache:
        nc = _build(groups, gt)
        # the jaxpr lowering re-serializes the module on every call; the
        # program is immutable after finalize(), so serialize once
        try:
            blob = nc.to_json_bytes()
            nc.to_json_bytes = lambda _b=blob: _b
        except Exception:
            pass
        _cache[key] = nc
    return _cache[key]


# ----------------------------------------------------------------------------
# Entry point
# ----------------------------------------------------------------------------
def make_program_and_inputs(x, edge_index, Ws, att_src, att_dst, biases):
    x = np.asarray(x, dtype=np.float32)
    Ws = np.asarray(Ws, dtype=np.float32)
    att_src = np.asarray(att_src, dtype=np.float32)
    att_dst = np.asarray(att_dst, dtype=np.float32)
    biases = np.asarray(biases, dtype=np.float32)

    groups, gt, srci, counts, order, perm, iperm = _prep_edges(edge_index)
    nc = _get_program(groups, gt)

    # per-core x shards in (window, pos) order, greedy block dealing
    m = np.arange(NSH)
    wm = m // 128
    xsh = []
    for c in range(NCORES):
        j = iperm[wm, c]
        q = (wm * NCORES + j) * 128 + (m % 128)
        xc = np.zeros((NSH, D), np.float32)
        real = q < N_NODES
        xc[real] = x[order[q[real]]]
        xsh.append(xc.astype(ml_dtypes.bfloat16))

    a2 = np.zeros((64, L, 16), np.float32)
    for l in range(L):
        for h in range(H):
            a2[h * C:(h + 1) * C, l, h] = att_src[l, h]
            a2[h * C:(h + 1) * C, l, 8 + h] = att_dst[l, h]
    wts = np.zeros((64, L, 80), np.float32)
    for l in range(L):
        wts[:, l, 0:64] = Ws[l]
        wts[:, l, 64:80] = Ws[l] @ a2[:, l, :]
    wts = wts.astype(ml_dtypes.bfloat16)
    bias = biases.reshape(1, L * 64).copy()

    in_maps = []
    for c in range(NCORES):
        blob = b"".join([xsh[c].tobytes(),
                         np.ascontiguousarray(srci[c]).tobytes(),
                         wts.tobytes(), bias.tobytes(),
                         np.ascontiguousarray(counts[c]).tobytes()])
        in_maps.append(dict(pack=np.frombuffer(blob, np.uint8).copy()))

    # output reassembly: out_full[order[q]] = res[core(q)][m(q)]
    q = np.arange(N_NODES)
    r = q // 128
    w = r // NCORES
    j = r % NCORES
    core_q = perm[w, j]
    m_q = w * 128 + (q % 128)
    return nc, in_maps, (order, core_q, m_q)


def assemble_output(res, meta):
    order, core_q, m_q = meta
    shards = [np.asarray(res.results[c]["out"]).astype(np.float32)
              for c in range(NCORES)]
    allout = np.stack(shards)                          # [8, 2560, 64]
    out = np.empty((N_NODES, D), np.float32)
    out[order] = allout[core_q, m_q]
    return out


def kernel(x, edge_index, Ws, att_src, att_dst, biases):
    from concourse.bass_utils import run_bass_kernel_spmd

    nc, in_maps, meta = make_program_and_inputs(
        x, edge_index, Ws, att_src, att_dst, biases)
    res = run_bass_kernel_spmd(nc, in_maps, core_ids=list(range(NCORES)))
    return assemble_output(res, meta)


# revision 10
# speedup vs baseline: 37.0978x; 1.0009x over previous
"""GAT (8-layer, 8-head) Trainium2 Bass kernel v3, 8-core SPMD.

Strategy (degree-partitioned edge layout; gather + reduce, no scatter matmuls):
- Host: add self-loops; sort nodes by in-degree (desc); blocks of 128 sorted
  nodes are degree-homogeneous.  Snake-deal blocks: block r -> window r//8,
  core (r%8 on even windows, 7-r%8 on odd) so per-core real edge counts
  balance.  Per (core, window): partition p = dst-local index, tile t =
  per-dst edge rank.  T[w] = max degree in window w (static, shared).
- Device, per layer: per-window pipeline: dma_gather 256B table rows by src
  (slot partition = dst-local), e = lrelu(s_src + s_dst[p]), ex = exp(e)
  bf16 into gathered cols 64:72, R = [ex*h | ex] in place, tensor_reduce
  over tiles -> out/z, finals -> x, then immediately project the window with
  the NEXT layer's fused [W | W@A2] and stage its table rows.  The table
  AllGather is split in two (windows [0,SPLIT) early, rest at layer end) and
  double-buffered (TAB0/TAB1) so it overlaps the tail windows' gathers.
- Padding slots point at a sentinel row with s_src = -1e30 (=> ex = 0).
  Trailing padding of each window's final gather chunk is trimmed per-core:
  those idxs are -1 (ucode skips them; per-core valid counts ride in the
  input pack and feed num_idxs_reg via value_load), and the possibly-skipped
  tail tiles are pre-memset to the sentinel pattern.
"""

import numpy as np
import ml_dtypes

# Persistent XLA compilation cache: run_bass_kernel_spmd re-traces a fresh
# closure per call, so without this every invocation recompiles the same HLO.
try:
    import jax
    jax.config.update("jax_compilation_cache_dir", "/tmp/jax_comp_cache")
    jax.config.update("jax_persistent_cache_min_entry_size_bytes", 0)
    jax.config.update("jax_persistent_cache_min_compile_time_secs", 0.0)
except Exception:
    pass
try:
    jax.config.update("bass_fast_dispatch", True)
except Exception:
    pass

N_NODES = 20000
N_EDGES = 640000
L, H, C = 8, 8, 8
D = H * C  # 64
NEG_SLOPE = 0.2

NCORES = 8
WIN = 128
WPC = 20                  # windows per core
NSH = WIN * WPC           # 2560 nodes per shard
NPAD = NCORES * NSH       # 20480
SENT = NPAD               # sentinel table row
TROWS = NPAD + 1

_cache = {}
SIM1 = False              # single-core variant (fake allgather)
GMAX = 1024               # idxs per dma_gather call (HW ring max; fixed by runtime)
DSCRATCH = 16384          # SBUF bytes/partition for SWDGE descriptor rings
TRIM = True               # per-core trailing-padding trim via -1 idxs
REGPROBE = False          # use count regs even with TRIM off (isolation probe)
AGSPLIT = True            # split table AllGather (early half mid-layer)
COLL0 = False             # ablation: local DMA instead of AllGather


def _chunks(nW):
    """Chunk a window's nW slots: remainder first so the FINAL chunk is a
    full GMAX (maximizes the per-core trailing trim headroom)."""
    rem = nW % GMAX
    out = []
    if rem:
        out.append((0, rem))
    for a in range(rem, nW, GMAX):
        out.append((a, a + GMAX))
    return out


def _regions_of(groups):
    """Window boundaries of the three table regions: [0,b1) gathered early
    (mid-layer), [b1,b2) and [b2,WPC) gathered at layer end (the last one
    covers only the final small window, shortening the layer-exit chain).
    Returns (b1, b2); boundaries are group bounds."""
    bounds = [w0 + wc for (w0, wc, _) in groups]
    b1 = min(bounds, key=lambda b: abs(b - (WPC - 4)))
    later = [b for b in bounds if b1 < b < WPC]
    b2 = max(later) if later else b1
    return b1, b2


# ----------------------------------------------------------------------------
# Host preprocessing
# ----------------------------------------------------------------------------
def _prep_edges(edge_index):
    src = np.asarray(edge_index[0], dtype=np.int64).astype(np.int32)
    dst = np.asarray(edge_index[1], dtype=np.int64).astype(np.int32)
    loops = np.arange(N_NODES, dtype=np.int32)
    src = np.concatenate([src, loops])
    dst = np.concatenate([dst, loops])

    deg = np.bincount(dst, minlength=N_NODES)          # incl. self loop
    order = np.argsort(-deg, kind="stable")            # orig ids, deg desc
    newid = np.empty(N_NODES, np.int32)
    newid[order] = np.arange(N_NODES, dtype=np.int32)

    degq = deg[order]                                   # deg sorted desc
    T = tuple(int(degq[w * 1024]) for w in range(WPC))
    assert all(t > 0 for t in T)
    # groups of adjacent equal-T windows (zero tile padding) sharing DVE ops
    groups = []                                          # (w0, wcount, Tg)
    w0 = 0
    while w0 < WPC:
        Tg = T[w0]
        w1 = w0 + 1
        while w1 < WPC and T[w1] == Tg:
            w1 += 1
        groups.append((w0, w1 - w0, Tg))
        w0 = w1
    groups = tuple(groups)
    b1, b2 = _regions_of(groups)
    off = np.zeros(WPC, np.int64)
    off[1:] = np.cumsum(T[:-1])
    nslot = int(sum(T)) * 128

    # greedy (LPT-ish) block dealing: per window, heaviest block to the
    # currently lightest core; perm[w, j] = core holding block 8w+j
    degq_pad = np.concatenate(
        [degq, np.zeros(NPAD - N_NODES, degq.dtype)])
    bsum = degq_pad.reshape(NCORES * WPC, 128).sum(1)
    loads = np.zeros(NCORES, np.int64)
    perm = np.zeros((WPC, NCORES), np.int32)
    for w in range(WPC):
        bs = bsum[NCORES * w:NCORES * (w + 1)]
        for bi in np.argsort(-bs):
            c = int(np.argmin(loads))
            perm[w, bi] = c
            loads[c] += bs[bi]
    iperm = np.zeros((WPC, NCORES), np.int32)   # iperm[w, c] = block j
    for w in range(WPC):
        iperm[w, perm[w]] = np.arange(NCORES)

    # table ids: three contiguous regions (AllGather fires per region)
    q_all = np.arange(NPAD, dtype=np.int64)
    r_all = q_all // 128
    w_all = r_all // NCORES
    j_all = r_all % NCORES
    c_all = perm[w_all, j_all].astype(np.int64)
    p_all = q_all % 128
    RB = (0, b1, b2, WPC)
    tab_of_q = np.zeros(NPAD, np.int64)
    base = 0
    for ri in range(3):
        lo, hi = RB[ri], RB[ri + 1]
        sz = hi - lo
        m = (w_all >= lo) & (w_all < hi)
        tab_of_q[m] = (base + c_all[m] * (sz * 128)
                       + (w_all[m] - lo) * 128 + p_all[m])
        base += NCORES * sz * 128
    tab_of_q = tab_of_q.astype(np.int32)

    qd = newid[dst]
    ts = tab_of_q[newid[src]].astype(np.int16)

    o2 = np.argsort(qd, kind="stable")
    qds = qd[o2]
    newgrp = np.empty(len(qds), bool)
    newgrp[0] = True
    newgrp[1:] = qds[1:] != qds[:-1]
    gidx = np.cumsum(newgrp) - 1
    gstart = np.flatnonzero(newgrp)
    t_rank = np.arange(len(qds), dtype=np.int64) - gstart[gidx]

    r_s = qds // 128
    wl_s = r_s // NCORES
    j_s = r_s % NCORES
    core_s = perm[wl_s, j_s]
    p_s = qds % 128
    pos = (off[wl_s] + t_rank) * 128 + p_s

    srcslot = np.full((NCORES, nslot), SENT, dtype=np.int16)
    srcslot[core_s, pos] = ts[o2]

    # per-core trailing-padding trim: within each window's FINAL gather
    # chunk, replace the trailing run of sentinels with -1 (ucode skips
    # trailing negatives); per-core valid counts ship in the pack.
    counts = np.zeros((NCORES, WPC), np.int32)
    gt = [0] * WPC
    for w in range(WPC):
        nW = T[w] * 128
        base = int(off[w]) * 128
        last_len = min(GMAX, nW)
        tmax = 0
        for c in range(NCORES):
            arr = srcslot[c, base:base + nW]
            nz = np.flatnonzero(arr != SENT)
            run = nW if len(nz) == 0 else nW - 1 - int(nz[-1])
            # keep >=128 valid idxs: an all-negative gather call is invalid
            trim = min(run, last_len - 128) if TRIM else 0
            if trim > 0:
                arr[nW - trim:nW] = -1
            counts[c, w] = last_len - trim
            tmax = max(tmax, trim)
        gt[w] = (tmax + 127) // 128
    gt = tuple(gt)

    # wrap16: flat k -> [k % 16, k // 16]
    srci = np.stack([srcslot[c].reshape(-1, 16).T.copy()
                     for c in range(NCORES)])            # [8, 16, nslot/16]

    return groups, gt, srci, counts, order, perm, iperm


# ----------------------------------------------------------------------------
# Bass program
# ----------------------------------------------------------------------------
def _build(groups, gt):
    import concourse.bass as bass
    import concourse.tile as tile
    import concourse.mybir as mybir
    from concourse import bacc
    from contextlib import ExitStack

    f32 = mybir.dt.float32
    bf16 = mybir.dt.bfloat16
    i16 = mybir.dt.int16
    i32 = mybir.dt.int32
    Alu = mybir.AluOpType
    Act = mybir.ActivationFunctionType
    Ax = mybir.AxisListType

    T = []
    for (_, wc, Tg) in groups:
        T += [Tg] * wc
    GTmax = max(wc * Tg for (_, wc, Tg) in groups)   # tiles in largest group
    off = [0]
    for t in T[:-1]:
        off.append(off[-1] + t)
    nslot = sum(T) * 128
    n16 = nslot // 16
    b1, b2 = _regions_of(groups)
    RB = (0, b1, b2, WPC)                         # region window bounds
    RSZ = [RB[i + 1] - RB[i] for i in range(3)]   # windows per region
    RBASE = [0]                                   # TAB row base per region
    for sz in RSZ[:-1]:
        RBASE.append(RBASE[-1] + NCORES * sz * 128)
    bounds = [w0 + wc for (w0, wc, _) in groups]
    # emit region-0's AllGather one group after its last window is staged,
    # so the issuing engine never waits on that window's reduce chain
    i1 = bounds.index(b1)
    ag0_emit = bounds[min(i1 + 1, len(bounds) - 1)]

    nc = bacc.Bacc("TRN2", target_bir_lowering=False, debug=False,
                   num_devices=1 if SIM1 else NCORES,
                   dynamic_dma_scratch_size=DSCRATCH)

    # packed input: [xsh bf16 | srci i16 | wts bf16 | bias f32 | counts i32]
    u8 = mybir.dt.uint8
    O_X = 0
    O_SRCI = O_X + NSH * D * 2
    O_WTS = O_SRCI + 16 * n16 * 2
    O_BIAS = O_WTS + 64 * L * 80 * 2
    O_CNT = O_BIAS + L * 64 * 4
    NB = O_CNT + WPC * 4
    t_pack = nc.dram_tensor("pack", [NB], u8, kind="ExternalInput")
    t_out = nc.dram_tensor("out", [NSH, D], bf16, kind="ExternalOutput")

    with tile.TileContext(nc) as tc, ExitStack() as ctx:
        cpool = ctx.enter_context(tc.tile_pool(name="const", bufs=1))
        wpool = ctx.enter_context(tc.tile_pool(name="work", bufs=2))
        gpool = ctx.enter_context(tc.tile_pool(name="gath", bufs=2))
        epool = ctx.enter_context(tc.tile_pool(name="edge", bufs=2))
        dram = ctx.enter_context(tc.tile_pool(name="dram", bufs=1, space="DRAM"))
        psT = ctx.enter_context(tc.tile_pool(name="psT", bufs=2, space="PSUM"))
        psA = ctx.enter_context(tc.tile_pool(name="psA", bufs=2, space="PSUM"))

        # persistent SBUF
        sb_xb = cpool.tile([128, WPC, D], bf16)
        sb_x = cpool.tile([128, WPC, D], f32)
        sb_srci = cpool.tile([128, n16], i16)
        sb_wts = cpool.tile([64, L, 80], bf16)
        sb_bias = cpool.tile([1, L * 64], f32)
        sb_brep = cpool.tile([128, L * 64], f32)
        sb_ident = cpool.tile([128, 128], f32)
        sb_cnt = cpool.tile([1, WPC], i32)
        tabsb = cpool.tile([128, WPC, 128], bf16)

        ap_x = t_pack.ap()[O_X:O_SRCI].bitcast(bf16)
        ap_srci = t_pack.ap()[O_SRCI:O_WTS].bitcast(i16)
        ap_wts = t_pack.ap()[O_WTS:O_BIAS].bitcast(bf16)
        ap_bias = t_pack.ap()[O_BIAS:O_CNT].bitcast(f32)
        ap_cnt = t_pack.ap()[O_CNT:NB].bitcast(i32)
        nc.sync.dma_start(sb_xb[:],
                          ap_x.rearrange("(t p c) -> p t c", p=128, c=D))
        nc.vector.tensor_copy(sb_x[:], sb_xb[:])
        for k in range(8):
            nc.sync.dma_start(sb_srci[16 * k:16 * (k + 1), :],
                              ap_srci.rearrange("(p c) -> p c", p=16))
        nc.sync.dma_start(sb_wts[:],
                          ap_wts.rearrange("(p l c) -> p l c", p=64, c=80))
        nc.sync.dma_start(sb_bias[:], ap_bias.rearrange("(p c) -> p c", p=1))
        nc.sync.dma_start(sb_cnt[:], ap_cnt.rearrange("(p c) -> p c", p=1))
        nc.gpsimd.partition_broadcast(sb_brep[:], sb_bias[:])
        # identity = affine_select(p - j == 0 ? 1 : 0)
        nc.vector.memset(sb_ident[:], 1.0)
        nc.gpsimd.affine_select(sb_ident[:], sb_ident[:], pattern=[[-1, 128]],
                                compare_op=Alu.is_equal, fill=0.0,
                                base=0, channel_multiplier=1)

        # per-window valid-count registers for the final gather chunks
        cnt_val = [None] * WPC
        if TRIM or REGPROBE:
            for w in range(WPC):
                r = nc.gpsimd.alloc_register(f"cntreg{w}")
                nc.gpsimd.reg_load(r, sb_cnt[0:1, w:w + 1])
                cnt_val[w] = r

        TAB = [dram.tile([TROWS, 128], bf16, name=f"TAB{i}") for i in (0, 1)]
        ST = [dram.tile([RSZ[i] * 128, 128], bf16, name=f"ST{i}")
              for i in range(3)]

        # cols 96:128 of table rows are never produced; zero once
        zjunk = cpool.tile([128, WPC, 32], bf16)
        nc.vector.memset(zjunk[:], 0.0)
        for i in range(3):
            nc.sync.dma_start(
                ST[i][:, 96:128].rearrange("(t p) c -> p t c", p=128),
                zjunk[:, 0:RSZ[i], :])

        # sentinel row: h=0, s_src=-1e30 => ex = 0 for padding slots
        sent = cpool.tile([1, 128], bf16)
        nc.vector.memset(sent[:], 0.0)
        nc.vector.memset(sent[:].bitcast(f32)[:, 32:40], -1e30)
        nc.sync.dma_start(TAB[0][SENT:SENT + 1, :], sent[:])
        nc.sync.dma_start(TAB[1][SENT:SENT + 1, :], sent[:])

        reg_cache = {}

        def nreg(n):
            if n not in reg_cache:
                reg_cache[n] = nc.gpsimd.to_reg(n)
            return reg_cache[n]

        def project_window(w, wl):
            """tabsb[:, w] <- table rows for layer `wl` from sb_x[:, w]."""
            ptx = psT.tile([64, 128], f32, tag="ptx")
            nc.tensor.transpose(ptx[:], sb_x[:, w, :], sb_ident[:])
            xTw = wpool.tile([64, 128], bf16, tag="xTw")
            nc.scalar.copy(xTw[:], ptx[:])
            ph = psA.tile([80, 128], f32, tag="ph")
            nc.tensor.matmul(ph[:], lhsT=sb_wts[:, wl, :], rhs=xTw[:],
                             start=True, stop=True)
            hsw = wpool.tile([80, 128], f32, tag="hsw")
            nc.scalar.copy(hsw[:], ph[:])
            ptb = psT.tile([128, 80], f32, tag="ptb")
            nc.tensor.transpose(ptb[:], hsw[:], sb_ident[:80, :80])
            nc.scalar.copy(tabsb[:, w, 0:64], ptb[:, 0:64])
            nc.vector.tensor_copy(
                tabsb[:, w:w + 1, :].bitcast(f32)[:, :, 32:48],
                ptb[:, 64:80].unsqueeze(1))

        def stage_window(w):
            ri = 0 if w < b1 else (1 if w < b2 else 2)
            ws = w - RB[ri]
            dst = ST[ri][ws * 128:(ws + 1) * 128, 0:96]
            nc.sync.dma_start(dst.rearrange("(t p) c -> p t c", p=128),
                              tabsb[:, w:w + 1, 0:96])

        def allgather(ri, tabw):
            lo = RBASE[ri]
            hi = lo + NCORES * RSZ[ri] * 128
            src = ST[ri]
            if SIM1 or COLL0:
                sz = RSZ[ri] * 128
                for c in range(NCORES):
                    nc.sync.dma_start(
                        tabw[lo + c * sz:lo + (c + 1) * sz, :], src[:])
            else:
                nc.gpsimd.collective_compute(
                    "AllGather", Alu.bypass,
                    replica_groups=[list(range(NCORES))],
                    ins=[src[:].opt()],
                    outs=[tabw[lo:hi, :].opt()],
                )

        # ---------------- prologue: table for layer 0 ----------------------
        for w in range(WPC):
            project_window(w, 0)
            stage_window(w)
        for ri in range(3):
            allgather(ri, TAB[0])

        # ---------------- layers -------------------------------------------
        for l in range(L):
            TABr = TAB[l % 2]
            TABw = TAB[(l + 1) % 2]
            layerbuf = wpool.tile([128, WPC, 72], f32, tag="layerbuf")
            for (w0, wc, Tg) in groups:
                tiles = wc * Tg
                vs = gpool.tile([128, GTmax, 128], bf16, tag="vs")
                # sentinel-pattern guard over possibly-skipped tail tiles
                for w in range(w0, w0 + wc):
                    if gt[w]:
                        a = (w - w0 + 1) * Tg - gt[w]
                        b = (w - w0 + 1) * Tg
                        nc.vector.memset(vs[:, a:b, :], 0.0)
                        nc.vector.memset(
                            vs[:, a:b, :].bitcast(f32)[:, :, 32:40], -1e30)
                # per-window gather calls (trailing -1s only in final chunk)
                for w in range(w0, w0 + wc):
                    nW = Tg * 128
                    i0 = 128 * off[w]
                    tb = (w - w0) * Tg
                    for (j0, j1) in _chunks(nW):
                        n = j1 - j0
                        reg = cnt_val[w] if ((TRIM or REGPROBE) and j1 == nW) else nreg(n)
                        nc.gpsimd.dma_gather(
                            out_ap=vs[:, tb + j0 // 128:tb + j1 // 128, :],
                            in_ap=TABr[:],
                            idxs_ap=sb_srci[:, (i0 + j0) // 16:(i0 + j1) // 16],
                            num_idxs=n, num_idxs_reg=reg, elem_size=128)

                e = epool.tile([128, GTmax, 8], f32, tag="e")
                nc.vector.tensor_tensor(
                    e[:, :tiles].rearrange("p (w t) c -> p w t c", w=wc),
                    vs[:, :tiles, :].bitcast(f32)[:, :, 32:40]
                        .rearrange("p (w t) c -> p w t c", w=wc),
                    tabsb[:, w0:w0 + wc, :].bitcast(f32)[:, :, 40:48]
                        .unsqueeze(2).broadcast_to([128, wc, Tg, 8]),
                    Alu.add)
                nc.vector.scalar_tensor_tensor(
                    e[:, :tiles], e[:, :tiles], NEG_SLOPE, e[:, :tiles],
                    op0=Alu.mult, op1=Alu.max)
                nc.scalar.activation(vs[:, :tiles, 64:72], e[:, :tiles],
                                     Act.Exp)
                nc.vector.tensor_tensor(
                    vs[:, :tiles, 0:64].rearrange("p t (h c) -> p t h c", h=8),
                    vs[:, :tiles, 0:64].rearrange("p t (h c) -> p t h c", h=8),
                    vs[:, :tiles, 64:72].unsqueeze(3).broadcast_to(
                        [128, tiles, 8, 8]),
                    Alu.mult)
                nc.vector.tensor_reduce(
                    layerbuf[:, w0:w0 + wc, :],
                    vs[:, :tiles, 0:72].rearrange("p (w t) c -> p w c t", w=wc),
                    axis=Ax.X, op=Alu.add)

                # finals for this group: x = out/(z+eps) + b
                zi = epool.tile([128, wc, 8], f32, tag=f"zi{wc}")
                nc.vector.tensor_scalar_add(
                    zi[:], layerbuf[:, w0:w0 + wc, 64:72], 1e-16)
                nc.vector.reciprocal(zi[:], zi[:])
                nc.vector.tensor_tensor(
                    sb_x[:, w0:w0 + wc].rearrange("p w (h c) -> p w h c", h=8),
                    layerbuf[:, w0:w0 + wc, 0:64]
                        .rearrange("p w (h c) -> p w h c", h=8),
                    zi[:].unsqueeze(3).broadcast_to([128, wc, 8, 8]),
                    Alu.mult)
                nc.vector.tensor_tensor(
                    sb_x[:, w0:w0 + wc], sb_x[:, w0:w0 + wc],
                    sb_brep[:, l * 64:(l + 1) * 64].unsqueeze(1)
                        .broadcast_to([128, wc, 64]),
                    Alu.add)

                if l < L - 1:
                    for w in range(w0, w0 + wc):
                        project_window(w, l + 1)
                        stage_window(w)
                    if w0 + wc == ag0_emit:
                        allgather(0, TABw)
            if l < L - 1:
                allgather(1, TABw)
                allgather(2, TABw)

        ob = cpool.tile([128, WPC, D], bf16)
        nc.vector.tensor_copy(ob[:], sb_x[:])
        nc.sync.dma_start(t_out.ap().rearrange("(t p) c -> p t c", p=128),
                          ob[:])

    nc.finalize()
    return nc


def _get_program(groups, gt):
    key = (groups, gt, SIM1, GMAX, COLL0, TRIM, REGPROBE, DSCRATCH)
    if key not in _cache:
        nc = _build(groups, gt)
        # the jaxpr lowering re-serializes the module on every call; the
        # program is immutable after finalize(), so serialize once
        try:
            blob = nc.to_json_bytes()
            nc.to_json_bytes = lambda _b=blob: _b
        except Exception:
            pass
        _cache[key] = nc
    return _cache[key]


# ----------------------------------------------------------------------------
# Entry point
# ----------------------------------------------------------------------------
def make_program_and_inputs(x, edge_index, Ws, att_src, att_dst, biases):
    x = np.asarray(x, dtype=np.float32)
    Ws = np.asarray(Ws, dtype=np.float32)
    att_src = np.asarray(att_src, dtype=np.float32)
    att_dst = np.asarray(att_dst, dtype=np.float32)
    biases = np.asarray(biases, dtype=np.float32)

    groups, gt, srci, counts, order, perm, iperm = _prep_edges(edge_index)
    nc = _get_program(groups, gt)

    # per-core x shards in (window, pos) order, greedy block dealing
    m = np.arange(NSH)
    wm = m // 128
    xsh = []
    for c in range(NCORES):
        j = iperm[wm, c]
        q = (wm * NCORES + j) * 128 + (m % 128)
        xc = np.zeros((NSH, D), np.float32)
        real = q < N_NODES
        xc[real] = x[order[q[real]]]
        xsh.append(xc.astype(ml_dtypes.bfloat16))

    a2 = np.zeros((64, L, 16), np.float32)
    for l in range(L):
        for h in range(H):
            a2[h * C:(h + 1) * C, l, h] = att_src[l, h]
            a2[h * C:(h + 1) * C, l, 8 + h] = att_dst[l, h]
    wts = np.zeros((64, L, 80), np.float32)
    for l in range(L):
        wts[:, l, 0:64] = Ws[l]
        wts[:, l, 64:80] = Ws[l] @ a2[:, l, :]
    wts = wts.astype(ml_dtypes.bfloat16)
    bias = biases.reshape(1, L * 64).copy()

    in_maps = []
    for c in range(NCORES):
        blob = b"".join([xsh[c].tobytes(),
                         np.ascontiguousarray(srci[c]).tobytes(),
                         wts.tobytes(), bias.tobytes(),
                         np.ascontiguousarray(counts[c]).tobytes()])
        in_maps.append(dict(pack=np.frombuffer(blob, np.uint8).copy()))

    # output reassembly: out_full[order[q]] = res[core(q)][m(q)]
    q = np.arange(N_NODES)
    r = q // 128
    w = r // NCORES
    j = r % NCORES
    core_q = perm[w, j]
    m_q = w * 128 + (q % 128)
    return nc, in_maps, (order, core_q, m_q)


def assemble_output(res, meta):
    order, core_q, m_q = meta
    shards = [np.asarray(res.results[c]["out"]).astype(np.float32)
              for c in range(NCORES)]
    allout = np.stack(shards)                          # [8, 2560, 64]
    out = np.empty((N_NODES, D), np.float32)
    out[order] = allout[core_q, m_q]
    return out


def kernel(x, edge_index, Ws, att_src, att_dst, biases):
    from concourse.bass_utils import run_bass_kernel_spmd

    nc, in_maps, meta = make_program_and_inputs(
        x, edge_index, Ws, att_src, att_dst, biases)
    res = run_bass_kernel_spmd(nc, in_maps, core_ids=list(range(NCORES)))
    return assemble_output(res, meta)


# revision 11
# speedup vs baseline: 37.3429x; 1.0066x over previous
"""GAT (8-layer, 8-head) Trainium2 Bass kernel v3, 8-core SPMD.

Strategy (degree-partitioned edge layout; gather + reduce, no scatter matmuls):
- Host: add self-loops; sort nodes by in-degree (desc); blocks of 128 sorted
  nodes are degree-homogeneous.  Snake-deal blocks: block r -> window r//8,
  core (r%8 on even windows, 7-r%8 on odd) so per-core real edge counts
  balance.  Per (core, window): partition p = dst-local index, tile t =
  per-dst edge rank.  T[w] = max degree in window w (static, shared).
- Device, per layer: per-window pipeline: dma_gather 256B table rows by src
  (slot partition = dst-local), e = lrelu(s_src + s_dst[p]), ex = exp(e)
  bf16 into gathered cols 64:72, R = [ex*h | ex] in place, tensor_reduce
  over tiles -> out/z, finals -> x, then immediately project the window with
  the NEXT layer's fused [W | W@A2] and stage its table rows.  The table
  AllGather is split in two (windows [0,SPLIT) early, rest at layer end) and
  double-buffered (TAB0/TAB1) so it overlaps the tail windows' gathers.
- Padding slots point at a sentinel row with s_src = -1e30 (=> ex = 0).
  Trailing padding of each window's final gather chunk is trimmed per-core:
  those idxs are -1 (ucode skips them; per-core valid counts ride in the
  input pack and feed num_idxs_reg via value_load), and the possibly-skipped
  tail tiles are pre-memset to the sentinel pattern.
"""

import numpy as np
import ml_dtypes

# Persistent XLA compilation cache: run_bass_kernel_spmd re-traces a fresh
# closure per call, so without this every invocation recompiles the same HLO.
try:
    import jax
    jax.config.update("jax_compilation_cache_dir", "/tmp/jax_comp_cache")
    jax.config.update("jax_persistent_cache_min_entry_size_bytes", 0)
    jax.config.update("jax_persistent_cache_min_compile_time_secs", 0.0)
except Exception:
    pass
try:
    jax.config.update("bass_fast_dispatch", True)
except Exception:
    pass

N_NODES = 20000
N_EDGES = 640000
L, H, C = 8, 8, 8
D = H * C  # 64
NEG_SLOPE = 0.2

NCORES = 8
WIN = 128
WPC = 20                  # windows per core
NSH = WIN * WPC           # 2560 nodes per shard
NPAD = NCORES * NSH       # 20480
SENT = NPAD               # sentinel table row
TROWS = NPAD + 1

_cache = {}
SIM1 = False              # single-core variant (fake allgather)
GMAX = 1024               # idxs per dma_gather call (HW ring max; fixed by runtime)
DSCRATCH = 16384          # SBUF bytes/partition for SWDGE descriptor rings
TRIM = True               # per-core trailing-padding trim via -1 idxs
REGPROBE = False          # use count regs even with TRIM off (isolation probe)
AGSPLIT = True            # split table AllGather (early half mid-layer)
COLL0 = False             # ablation: local DMA instead of AllGather


def _chunks(nW):
    """Chunk a window's nW slots: remainder first so the FINAL chunk is a
    full GMAX (maximizes the per-core trailing trim headroom)."""
    rem = nW % GMAX
    out = []
    if rem:
        out.append((0, rem))
    for a in range(rem, nW, GMAX):
        out.append((a, a + GMAX))
    return out


def _regions_of(groups):
    """Window boundaries of the three table regions: [0,b1) gathered early
    (mid-layer), [b1,b2) and [b2,WPC) gathered at layer end (the last one
    covers only the final small window, shortening the layer-exit chain).
    Returns (b1, b2); boundaries are group bounds."""
    bounds = [w0 + wc for (w0, wc, _) in groups]
    b1 = min(bounds, key=lambda b: abs(b - (WPC - 4)))
    later = [b for b in bounds if b1 < b < WPC]
    b2 = max(later) if later else b1
    return b1, b2


# ----------------------------------------------------------------------------
# Host preprocessing
# ----------------------------------------------------------------------------
def _prep_edges(edge_index):
    src = np.asarray(edge_index[0], dtype=np.int64).astype(np.int32)
    dst = np.asarray(edge_index[1], dtype=np.int64).astype(np.int32)
    loops = np.arange(N_NODES, dtype=np.int32)
    src = np.concatenate([src, loops])
    dst = np.concatenate([dst, loops])

    deg = np.bincount(dst, minlength=N_NODES)          # incl. self loop
    order = np.argsort(-deg, kind="stable")            # orig ids, deg desc
    newid = np.empty(N_NODES, np.int32)
    newid[order] = np.arange(N_NODES, dtype=np.int32)

    degq = deg[order]                                   # deg sorted desc
    T = tuple(int(degq[w * 1024]) for w in range(WPC))
    assert all(t > 0 for t in T)
    # groups of adjacent equal-T windows (zero tile padding) sharing DVE ops
    groups = []                                          # (w0, wcount, Tg)
    w0 = 0
    while w0 < WPC:
        Tg = T[w0]
        w1 = w0 + 1
        while w1 < WPC and T[w1] == Tg:
            w1 += 1
        groups.append((w0, w1 - w0, Tg))
        w0 = w1
    groups = tuple(groups)
    b1, b2 = _regions_of(groups)
    off = np.zeros(WPC, np.int64)
    off[1:] = np.cumsum(T[:-1])
    nslot = int(sum(T)) * 128

    # greedy (LPT-ish) block dealing: per window, heaviest block to the
    # currently lightest core; perm[w, j] = core holding block 8w+j
    degq_pad = np.concatenate(
        [degq, np.zeros(NPAD - N_NODES, degq.dtype)])
    bsum = degq_pad.reshape(NCORES * WPC, 128).sum(1)
    loads = np.zeros(NCORES, np.int64)
    perm = np.zeros((WPC, NCORES), np.int32)
    for w in range(WPC):
        bs = bsum[NCORES * w:NCORES * (w + 1)]
        for bi in np.argsort(-bs):
            c = int(np.argmin(loads))
            perm[w, bi] = c
            loads[c] += bs[bi]
    iperm = np.zeros((WPC, NCORES), np.int32)   # iperm[w, c] = block j
    for w in range(WPC):
        iperm[w, perm[w]] = np.arange(NCORES)

    # table ids: three contiguous regions (AllGather fires per region)
    q_all = np.arange(NPAD, dtype=np.int64)
    r_all = q_all // 128
    w_all = r_all // NCORES
    j_all = r_all % NCORES
    c_all = perm[w_all, j_all].astype(np.int64)
    p_all = q_all % 128
    RB = (0, b1, b2, WPC)
    tab_of_q = np.zeros(NPAD, np.int64)
    base = 0
    for ri in range(3):
        lo, hi = RB[ri], RB[ri + 1]
        sz = hi - lo
        m = (w_all >= lo) & (w_all < hi)
        tab_of_q[m] = (base + c_all[m] * (sz * 128)
                       + (w_all[m] - lo) * 128 + p_all[m])
        base += NCORES * sz * 128
    tab_of_q = tab_of_q.astype(np.int32)

    qd = newid[dst]
    ts = tab_of_q[newid[src]].astype(np.int16)

    o2 = np.argsort(qd, kind="stable")
    qds = qd[o2]
    newgrp = np.empty(len(qds), bool)
    newgrp[0] = True
    newgrp[1:] = qds[1:] != qds[:-1]
    gidx = np.cumsum(newgrp) - 1
    gstart = np.flatnonzero(newgrp)
    t_rank = np.arange(len(qds), dtype=np.int64) - gstart[gidx]

    r_s = qds // 128
    wl_s = r_s // NCORES
    j_s = r_s % NCORES
    core_s = perm[wl_s, j_s]
    p_s = qds % 128
    pos = (off[wl_s] + t_rank) * 128 + p_s

    srcslot = np.full((NCORES, nslot), SENT, dtype=np.int16)
    srcslot[core_s, pos] = ts[o2]

    # per-core trailing-padding trim: within each window's FINAL gather
    # chunk, replace the trailing run of sentinels with -1 (ucode skips
    # trailing negatives); per-core valid counts ship in the pack.
    counts = np.zeros((NCORES, WPC), np.int32)
    gt = [0] * WPC
    for w in range(WPC):
        nW = T[w] * 128
        base = int(off[w]) * 128
        last_len = min(GMAX, nW)
        tmax = 0
        for c in range(NCORES):
            arr = srcslot[c, base:base + nW]
            nz = np.flatnonzero(arr != SENT)
            run = nW if len(nz) == 0 else nW - 1 - int(nz[-1])
            # keep >=128 valid idxs: an all-negative gather call is invalid
            trim = min(run, last_len - 128) if TRIM else 0
            if trim > 0:
                arr[nW - trim:nW] = -1
            counts[c, w] = last_len - trim
            tmax = max(tmax, trim)
        gt[w] = (tmax + 127) // 128
    gt = tuple(gt)

    # wrap16: flat k -> [k % 16, k // 16]
    srci = np.stack([srcslot[c].reshape(-1, 16).T.copy()
                     for c in range(NCORES)])            # [8, 16, nslot/16]

    return groups, gt, srci, counts, order, perm, iperm


# ----------------------------------------------------------------------------
# Bass program
# ----------------------------------------------------------------------------
def _build(groups, gt):
    import concourse.bass as bass
    import concourse.tile as tile
    import concourse.mybir as mybir
    from concourse import bacc
    from contextlib import ExitStack

    f32 = mybir.dt.float32
    bf16 = mybir.dt.bfloat16
    i16 = mybir.dt.int16
    i32 = mybir.dt.int32
    Alu = mybir.AluOpType
    Act = mybir.ActivationFunctionType
    Ax = mybir.AxisListType

    T = []
    for (_, wc, Tg) in groups:
        T += [Tg] * wc
    GTmax = max(wc * Tg for (_, wc, Tg) in groups)   # tiles in largest group
    off = [0]
    for t in T[:-1]:
        off.append(off[-1] + t)
    nslot = sum(T) * 128
    n16 = nslot // 16
    b1, b2 = _regions_of(groups)
    RB = (0, b1, b2, WPC)                         # region window bounds
    RSZ = [RB[i + 1] - RB[i] for i in range(3)]   # windows per region
    RBASE = [0]                                   # TAB row base per region
    for sz in RSZ[:-1]:
        RBASE.append(RBASE[-1] + NCORES * sz * 128)
    bounds = [w0 + wc for (w0, wc, _) in groups]
    # emit region-0's AllGather one group after its last window is staged,
    # so the issuing engine never waits on that window's reduce chain
    i1 = bounds.index(b1)
    ag0_emit = bounds[min(i1 + 1, len(bounds) - 1)]

    nc = bacc.Bacc("TRN2", target_bir_lowering=False, debug=False,
                   num_devices=1 if SIM1 else NCORES,
                   dynamic_dma_scratch_size=DSCRATCH)

    # packed input: [xsh bf16 | srci i16 | wts bf16 | bias f32 | counts i32]
    u8 = mybir.dt.uint8
    O_X = 0
    O_SRCI = O_X + NSH * D * 2
    O_WTS = O_SRCI + 16 * n16 * 2
    O_BIAS = O_WTS + 64 * L * 80 * 2
    O_CNT = O_BIAS + L * 64 * 4
    NB = O_CNT + WPC * 4
    t_pack = nc.dram_tensor("pack", [NB], u8, kind="ExternalInput")
    t_out = nc.dram_tensor("out", [NSH, D], bf16, kind="ExternalOutput")

    with tile.TileContext(nc) as tc, ExitStack() as ctx:
        cpool = ctx.enter_context(tc.tile_pool(name="const", bufs=1))
        wpool = ctx.enter_context(tc.tile_pool(name="work", bufs=2))
        gpool = ctx.enter_context(tc.tile_pool(name="gath", bufs=3))
        epool = ctx.enter_context(tc.tile_pool(name="edge", bufs=2))
        dram = ctx.enter_context(tc.tile_pool(name="dram", bufs=1, space="DRAM"))
        psT = ctx.enter_context(tc.tile_pool(name="psT", bufs=2, space="PSUM"))
        psA = ctx.enter_context(tc.tile_pool(name="psA", bufs=2, space="PSUM"))

        # persistent SBUF
        sb_xb = cpool.tile([128, WPC, D], bf16)
        sb_x = cpool.tile([128, WPC, D], f32)
        sb_srci = cpool.tile([128, n16], i16)
        sb_wts = cpool.tile([64, L, 80], bf16)
        sb_bias = cpool.tile([1, L * 64], f32)
        sb_brep = cpool.tile([128, L * 64], f32)
        sb_ident = cpool.tile([128, 128], f32)
        sb_cnt = cpool.tile([1, WPC], i32)
        tabsb = cpool.tile([128, WPC, 128], bf16)

        ap_x = t_pack.ap()[O_X:O_SRCI].bitcast(bf16)
        ap_srci = t_pack.ap()[O_SRCI:O_WTS].bitcast(i16)
        ap_wts = t_pack.ap()[O_WTS:O_BIAS].bitcast(bf16)
        ap_bias = t_pack.ap()[O_BIAS:O_CNT].bitcast(f32)
        ap_cnt = t_pack.ap()[O_CNT:NB].bitcast(i32)
        nc.sync.dma_start(sb_xb[:],
                          ap_x.rearrange("(t p c) -> p t c", p=128, c=D))
        nc.vector.tensor_copy(sb_x[:], sb_xb[:])
        for k in range(8):
            nc.sync.dma_start(sb_srci[16 * k:16 * (k + 1), :],
                              ap_srci.rearrange("(p c) -> p c", p=16))
        nc.sync.dma_start(sb_wts[:],
                          ap_wts.rearrange("(p l c) -> p l c", p=64, c=80))
        nc.sync.dma_start(sb_bias[:], ap_bias.rearrange("(p c) -> p c", p=1))
        nc.sync.dma_start(sb_cnt[:], ap_cnt.rearrange("(p c) -> p c", p=1))
        nc.gpsimd.partition_broadcast(sb_brep[:], sb_bias[:])
        # identity = affine_select(p - j == 0 ? 1 : 0)
        nc.vector.memset(sb_ident[:], 1.0)
        nc.gpsimd.affine_select(sb_ident[:], sb_ident[:], pattern=[[-1, 128]],
                                compare_op=Alu.is_equal, fill=0.0,
                                base=0, channel_multiplier=1)

        # per-window valid-count registers for the final gather chunks
        cnt_val = [None] * WPC
        if TRIM or REGPROBE:
            for w in range(WPC):
                r = nc.gpsimd.alloc_register(f"cntreg{w}")
                nc.gpsimd.reg_load(r, sb_cnt[0:1, w:w + 1])
                cnt_val[w] = r

        TAB = [dram.tile([TROWS, 128], bf16, name=f"TAB{i}") for i in (0, 1)]
        ST = [dram.tile([RSZ[i] * 128, 128], bf16, name=f"ST{i}")
              for i in range(3)]

        # cols 96:128 of table rows are never produced; zero once
        zjunk = cpool.tile([128, WPC, 32], bf16)
        nc.vector.memset(zjunk[:], 0.0)
        for i in range(3):
            nc.sync.dma_start(
                ST[i][:, 96:128].rearrange("(t p) c -> p t c", p=128),
                zjunk[:, 0:RSZ[i], :])

        # sentinel row: h=0, s_src=-1e30 => ex = 0 for padding slots
        sent = cpool.tile([1, 128], bf16)
        nc.vector.memset(sent[:], 0.0)
        nc.vector.memset(sent[:].bitcast(f32)[:, 32:40], -1e30)
        nc.sync.dma_start(TAB[0][SENT:SENT + 1, :], sent[:])
        nc.sync.dma_start(TAB[1][SENT:SENT + 1, :], sent[:])

        reg_cache = {}

        def nreg(n):
            if n not in reg_cache:
                reg_cache[n] = nc.gpsimd.to_reg(n)
            return reg_cache[n]

        def project_window(w, wl):
            """tabsb[:, w] <- table rows for layer `wl` from sb_x[:, w]."""
            ptx = psT.tile([64, 128], f32, tag="ptx")
            nc.tensor.transpose(ptx[:], sb_x[:, w, :], sb_ident[:])
            xTw = wpool.tile([64, 128], bf16, tag="xTw")
            nc.scalar.copy(xTw[:], ptx[:])
            ph = psA.tile([80, 128], f32, tag="ph")
            nc.tensor.matmul(ph[:], lhsT=sb_wts[:, wl, :], rhs=xTw[:],
                             start=True, stop=True)
            hsw = wpool.tile([80, 128], f32, tag="hsw")
            nc.scalar.copy(hsw[:], ph[:])
            ptb = psT.tile([128, 80], f32, tag="ptb")
            nc.tensor.transpose(ptb[:], hsw[:], sb_ident[:80, :80])
            nc.scalar.copy(tabsb[:, w, 0:64], ptb[:, 0:64])
            nc.vector.tensor_copy(
                tabsb[:, w:w + 1, :].bitcast(f32)[:, :, 32:48],
                ptb[:, 64:80].unsqueeze(1))

        def stage_window(w):
            ri = 0 if w < b1 else (1 if w < b2 else 2)
            ws = w - RB[ri]
            dst = ST[ri][ws * 128:(ws + 1) * 128, 0:96]
            nc.sync.dma_start(dst.rearrange("(t p) c -> p t c", p=128),
                              tabsb[:, w:w + 1, 0:96])

        def allgather(ri, tabw):
            lo = RBASE[ri]
            hi = lo + NCORES * RSZ[ri] * 128
            src = ST[ri]
            if SIM1 or COLL0:
                sz = RSZ[ri] * 128
                for c in range(NCORES):
                    nc.sync.dma_start(
                        tabw[lo + c * sz:lo + (c + 1) * sz, :], src[:])
            else:
                nc.gpsimd.collective_compute(
                    "AllGather", Alu.bypass,
                    replica_groups=[list(range(NCORES))],
                    ins=[src[:].opt()],
                    outs=[tabw[lo:hi, :].opt()],
                )

        # ---------------- prologue: table for layer 0 ----------------------
        for w in range(WPC):
            project_window(w, 0)
            stage_window(w)
        for ri in range(3):
            allgather(ri, TAB[0])

        # ---------------- layers -------------------------------------------
        for l in range(L):
            TABr = TAB[l % 2]
            TABw = TAB[(l + 1) % 2]
            layerbuf = wpool.tile([128, WPC, 72], f32, tag="layerbuf")
            for (w0, wc, Tg) in groups:
                tiles = wc * Tg
                vs = gpool.tile([128, GTmax, 128], bf16, tag="vs")
                # sentinel-pattern guard over possibly-skipped tail tiles
                for w in range(w0, w0 + wc):
                    if gt[w]:
                        a = (w - w0 + 1) * Tg - gt[w]
                        b = (w - w0 + 1) * Tg
                        nc.vector.memset(vs[:, a:b, :], 0.0)
                        nc.vector.memset(
                            vs[:, a:b, :].bitcast(f32)[:, :, 32:40], -1e30)
                # per-window gather calls (trailing -1s only in final chunk)
                for w in range(w0, w0 + wc):
                    nW = Tg * 128
                    i0 = 128 * off[w]
                    tb = (w - w0) * Tg
                    for (j0, j1) in _chunks(nW):
                        n = j1 - j0
                        reg = cnt_val[w] if ((TRIM or REGPROBE) and j1 == nW) else nreg(n)
                        nc.gpsimd.dma_gather(
                            out_ap=vs[:, tb + j0 // 128:tb + j1 // 128, :],
                            in_ap=TABr[:],
                            idxs_ap=sb_srci[:, (i0 + j0) // 16:(i0 + j1) // 16],
                            num_idxs=n, num_idxs_reg=reg, elem_size=128)

                e = epool.tile([128, GTmax, 8], f32, tag="e")
                nc.vector.tensor_tensor(
                    e[:, :tiles].rearrange("p (w t) c -> p w t c", w=wc),
                    vs[:, :tiles, :].bitcast(f32)[:, :, 32:40]
                        .rearrange("p (w t) c -> p w t c", w=wc),
                    tabsb[:, w0:w0 + wc, :].bitcast(f32)[:, :, 40:48]
                        .unsqueeze(2).broadcast_to([128, wc, Tg, 8]),
                    Alu.add)
                nc.vector.scalar_tensor_tensor(
                    e[:, :tiles], e[:, :tiles], NEG_SLOPE, e[:, :tiles],
                    op0=Alu.mult, op1=Alu.max)
                nc.scalar.activation(vs[:, :tiles, 64:72], e[:, :tiles],
                                     Act.Exp)
                nc.vector.tensor_tensor(
                    vs[:, :tiles, 0:64].rearrange("p t (h c) -> p t h c", h=8),
                    vs[:, :tiles, 0:64].rearrange("p t (h c) -> p t h c", h=8),
                    vs[:, :tiles, 64:72].unsqueeze(3).broadcast_to(
                        [128, tiles, 8, 8]),
                    Alu.mult)
                nc.vector.tensor_reduce(
                    layerbuf[:, w0:w0 + wc, :],
                    vs[:, :tiles, 0:72].rearrange("p (w t) c -> p w c t", w=wc),
                    axis=Ax.X, op=Alu.add)

                # finals for this group: x = out/(z+eps) + b
                zi = epool.tile([128, wc, 8], f32, tag=f"zi{wc}")
                nc.vector.tensor_scalar_add(
                    zi[:], layerbuf[:, w0:w0 + wc, 64:72], 1e-16)
                nc.vector.reciprocal(zi[:], zi[:])
                nc.vector.tensor_tensor(
                    sb_x[:, w0:w0 + wc].rearrange("p w (h c) -> p w h c", h=8),
                    layerbuf[:, w0:w0 + wc, 0:64]
                        .rearrange("p w (h c) -> p w h c", h=8),
                    zi[:].unsqueeze(3).broadcast_to([128, wc, 8, 8]),
                    Alu.mult)
                nc.vector.tensor_tensor(
                    sb_x[:, w0:w0 + wc], sb_x[:, w0:w0 + wc],
                    sb_brep[:, l * 64:(l + 1) * 64].unsqueeze(1)
                        .broadcast_to([128, wc, 64]),
                    Alu.add)

                if l < L - 1:
                    for w in range(w0, w0 + wc):
                        project_window(w, l + 1)
                        stage_window(w)
                    if w0 + wc == ag0_emit:
                        allgather(0, TABw)
            if l < L - 1:
                allgather(1, TABw)
                allgather(2, TABw)

        ob = cpool.tile([128, WPC, D], bf16)
        nc.vector.tensor_copy(ob[:], sb_x[:])
        nc.sync.dma_start(t_out.ap().rearrange("(t p) c -> p t c", p=128),
                          ob[:])

    nc.finalize()
    return nc


def _get_program(groups, gt):
    key = (groups, gt, SIM1, GMAX, COLL0, TRIM, REGPROBE, DSCRATCH)
    if key not in _cache:
        nc = _build(groups, gt)
        # the jaxpr lowering re-serializes the module on every call; the
        # program is immutable after finalize(), so serialize once
        try:
            blob = nc.to_json_bytes()
            nc.to_json_bytes = lambda _b=blob: _b
        except Exception:
            pass
        _cache[key] = nc
    return _cache[key]


# ----------------------------------------------------------------------------
# Entry point
# ----------------------------------------------------------------------------
def make_program_and_inputs(x, edge_index, Ws, att_src, att_dst, biases):
    x = np.asarray(x, dtype=np.float32)
    Ws = np.asarray(Ws, dtype=np.float32)
    att_src = np.asarray(att_src, dtype=np.float32)
    att_dst = np.asarray(att_dst, dtype=np.float32)
    biases = np.asarray(biases, dtype=np.float32)

    groups, gt, srci, counts, order, perm, iperm = _prep_edges(edge_index)
    nc = _get_program(groups, gt)

    # per-core x shards in (window, pos) order, greedy block dealing
    m = np.arange(NSH)
    wm = m // 128
    xsh = []
    for c in range(NCORES):
        j = iperm[wm, c]
        q = (wm * NCORES + j) * 128 + (m % 128)
        xc = np.zeros((NSH, D), np.float32)
        real = q < N_NODES
        xc[real] = x[order[q[real]]]
        xsh.append(xc.astype(ml_dtypes.bfloat16))

    a2 = np.zeros((64, L, 16), np.float32)
    for l in range(L):
        for h in range(H):
            a2[h * C:(h + 1) * C, l, h] = att_src[l, h]
            a2[h * C:(h + 1) * C, l, 8 + h] = att_dst[l, h]
    wts = np.zeros((64, L, 80), np.float32)
    for l in range(L):
        wts[:, l, 0:64] = Ws[l]
        wts[:, l, 64:80] = Ws[l] @ a2[:, l, :]
    wts = wts.astype(ml_dtypes.bfloat16)
    bias = biases.reshape(1, L * 64).copy()

    in_maps = []
    for c in range(NCORES):
        blob = b"".join([xsh[c].tobytes(),
                         np.ascontiguousarray(srci[c]).tobytes(),
                         wts.tobytes(), bias.tobytes(),
                         np.ascontiguousarray(counts[c]).tobytes()])
        in_maps.append(dict(pack=np.frombuffer(blob, np.uint8).copy()))

    # output reassembly: out_full[order[q]] = res[core(q)][m(q)]
    q = np.arange(N_NODES)
    r = q // 128
    w = r // NCORES
    j = r % NCORES
    core_q = perm[w, j]
    m_q = w * 128 + (q % 128)
    return nc, in_maps, (order, core_q, m_q)


def assemble_output(res, meta):
    order, core_q, m_q = meta
    shards = [np.asarray(res.results[c]["out"]).astype(np.float32)
              for c in range(NCORES)]
    allout = np.stack(shards)                          # [8, 2560, 64]
    out = np.empty((N_NODES, D), np.float32)
    out[order] = allout[core_q, m_q]
    return out


def kernel(x, edge_index, Ws, att_src, att_dst, biases):
    from concourse.bass_utils import run_bass_kernel_spmd

    nc, in_maps, meta = make_program_and_inputs(
        x, edge_index, Ws, att_src, att_dst, biases)
    res = run_bass_kernel_spmd(nc, in_maps, core_ids=list(range(NCORES)))
    return assemble_output(res, meta)
